# revision 9
# baseline (speedup 1.0000x reference)
"""AttentionBlock (GroupNorm -> qkv 1x1 -> 4-head attention over 4096 tokens
-> proj 1x1 -> residual) distributed over 8 TRN2 NeuronCores.

Sharding: zero-communication query sharding. Core j handles batch b = j//2 and
query half qh = j%2 (2048 of the 4096 spatial positions). Each core loads the
full x[b] (256, 4096), computes GroupNorm + K/V over all keys, Q only for its
2048 queries, and writes its (256, 2048) output slice.

Structure (v2, tuned for Act-engine saturation):
  - scores are computed transposed ([keys, queries] via lhsT=k, rhs=q) so the
    exp() output feeds the AV matmul as the *moving* operand: AV uses V^T as
    the stationary tensor (lhsT = [keys, d+1] per head, with a ones column
    appended so the softmax denominator accumulates as psum row 64) and
    streams 512-query tiles of exp(scores). This removes the per-tile
    LDWEIGHTS of exp(scores) and the output transposes of the previous
    oT-form, and leaves attention output directly in [channels, queries].
  - softmax max-subtraction is skipped (logits are O(4) std-normal, far from
    fp32/bf16 exp overflow); per-query normalization is applied after AV via
    an approx reciprocal of the denominator row, partition-broadcast to the
    64 head channels.
  - one exp() per [128 keys, 2 heads x 512 queries] psum tile; the j-loop
    covers 4 heads flat so PSUM is exactly 8 banks: 2x[128,1024] f32 score
    tiles (double buffered) + 4x[128,512] AV accumulators (the proj psum
    recycles accumulator banks).
  - the prologue is pipelined: x is DMA'd in chunks with bn_stats emitted per
    chunk, and K/V production for key-chunk n+1 is interleaved into the main
    loop of the first query chunk, so the Act engine starts exp'ing as early
    as possible.
  - matmuls run in bf16 (full 128x128 PE, K=128 zero-padded per-head K
    tensors); GroupNorm stats, softmax accumulation/normalization and the
    residual stay fp32.
"""

import numpy as np

import concourse.bass as bass
import concourse.tile as tile
from concourse import bacc, mybir
from concourse.bass_utils import run_bass_kernel_spmd

C = 256
HW = 4096
NH = 4
D = 64  # head dim
G = 8  # groups
EPS = 1e-5
SCALE = D**-0.5
Q = HW // 2  # queries per core
NJT = HW // 128  # 32 key tiles
NKC = 8  # key chunks (512 keys each) for K/V production
NIC = Q // 512  # 4 query chunks of 512

F32 = mybir.dt.float32
BF16 = mybir.dt.bfloat16


def build(finalize=True):
    nc = bacc.Bacc("TRN2", target_bir_lowering=False, debug=False, num_devices=8)

    x = nc.declare_dram_parameter("x", [C, HW], F32, isOutput=False)
    xq = nc.declare_dram_parameter("xq", [C, Q], F32, isOutput=False)
    wn2 = nc.declare_dram_parameter("wn2", [128, 2], F32, isOutput=False)
    bn2 = nc.declare_dram_parameter("bn2", [128, 2], F32, isOutput=False)
    wq = nc.declare_dram_parameter("wq", [128, 2, C], BF16, isOutput=False)
    bq2 = nc.declare_dram_parameter("bq2", [128, 2], F32, isOutput=False)
    wkz = nc.declare_dram_parameter("wkz", [128, 2, NH, 128], BF16, isOutput=False)
    bkz = nc.declare_dram_parameter("bkz", [128, NH], F32, isOutput=False)
    wv = nc.declare_dram_parameter("wv", [128, 2, NH * 65], BF16, isOutput=False)
    vb = nc.declare_dram_parameter("vb", [128, NH * 65], F32, isOutput=False)
    wproj = nc.declare_dram_parameter("wproj", [65, NH, 2, 128], BF16, isOutput=False)
    bproj2 = nc.declare_dram_parameter("bproj2", [128, 2], F32, isOutput=False)
    gmask = nc.declare_dram_parameter("gmask", [128, 2, 128], F32, isOutput=False)
    gmaskT = nc.declare_dram_parameter("gmaskT", [128, 2, 128], F32, isOutput=False)
    out = nc.declare_dram_parameter("out", [C, Q], F32, isOutput=True)

    Exp = mybir.ActivationFunctionType.Exp
    Ln = mybir.ActivationFunctionType.Ln
    Alu = mybir.AluOpType

    with tile.TileContext(nc) as tc:
        with (
            tc.tile_pool(name="keep", bufs=1) as keep,
            tc.tile_pool(name="consts", bufs=1) as consts,
            tc.tile_pool(name="small", bufs=4) as small,
            tc.tile_pool(name="s_ps", bufs=2, space="PSUM") as s_ps,
            tc.tile_pool(name="acc_ps", bufs=1, space="PSUM") as acc_ps,
            tc.tile_pool(name="exps", bufs=3) as expp,
            tc.tile_pool(name="att", bufs=2) as att,
        ):
            # ---- constants ----
            WQ = consts.tile([128, 2, C], BF16)
            nc.sync.dma_start(out=WQ, in_=wq[:])
            WKZ = consts.tile([128, 2, NH, 128], BF16)
            nc.sync.dma_start(out=WKZ, in_=wkz[:])
            BKZ = consts.tile([128, NH], F32)
            nc.sync.dma_start(out=BKZ, in_=bkz[:])
            WV = consts.tile([128, 2, NH * 65], BF16)
            nc.sync.dma_start(out=WV, in_=wv[:])
            WP = consts.tile([65, NH, 2, 128], BF16)
            nc.sync.dma_start(out=WP, in_=wproj[:])
            WN = consts.tile([128, 2], F32)
            nc.sync.dma_start(out=WN, in_=wn2[:])
            BN = consts.tile([128, 2], F32)
            nc.sync.dma_start(out=BN, in_=bn2[:])
            BQ = consts.tile([128, 2], F32)
            nc.sync.dma_start(out=BQ, in_=bq2[:])
            VB = consts.tile([128, NH * 65], F32)
            nc.sync.dma_start(out=VB, in_=vb[:])
            BP = consts.tile([128, 2], F32)
            nc.sync.dma_start(out=BP, in_=bproj2[:])
            GM = consts.tile([128, 2, 128], F32)
            nc.sync.dma_start(out=GM, in_=gmask[:])
            GMT = consts.tile([128, 2, 128], F32)
            nc.sync.dma_start(out=GMT, in_=gmaskT[:])
            EPS8 = consts.tile([G, 1], F32)
            nc.vector.memset(EPS8, EPS)

            XQ = [
                keep.tile([128, Q], F32, tag=f"XQ{t}", name=f"XQ{t}")
                for t in range(2)
            ]
            for t in range(2):
                nc.sync.dma_start(out=XQ[t], in_=xq[t * 128 : (t + 1) * 128, :])

            # persistent attention operands
            KZ = [
                keep.tile([128, HW], BF16, tag=f"KZ{h}", name=f"KZ{h}")
                for h in range(NH)
            ]
            QT = [
                keep.tile([128, Q], BF16, tag=f"Q{t}", name=f"Q{t}")
                for t in range(2)
            ]
            # V^T with a ones column per head: [keys, (head, d+1)]
            V = keep.tile([128, NJT, NH * 65], BF16)

            with tc.tile_pool(name="xh", bufs=1) as xh:
                X = [
                    xh.tile([128, HW], F32, tag=f"X{t}", name=f"X{t}")
                    for t in range(2)
                ]
                H = [
                    xh.tile([128, HW], BF16, tag=f"H{t}", name=f"H{t}")
                    for t in range(2)
                ]
                HQ = [
                    xh.tile([128, Q], BF16, tag=f"HQ{t}", name=f"HQ{t}")
                    for t in range(2)
                ]

                # ---- x DMA in chunks, bn_stats per chunk ----
                st = [
                    small.tile([128, 8, 6], F32, tag=f"bnst{t}", name=f"bnst{t}")
                    for t in range(2)
                ]
                for ch in range(4):
                    for t in range(2):
                        nc.sync.dma_start(
                            out=X[t][:, ch * 1024 : (ch + 1) * 1024],
                            in_=x[t * 128 : (t + 1) * 128, ch * 1024 : (ch + 1) * 1024],
                        )
                        xr = X[t].rearrange("p (n f) -> p n f", f=512)
                        for s in (2 * ch, 2 * ch + 1):
                            nc.vector.bn_stats(out=st[t][:, s], in_=xr[:, s])

                # ---- GroupNorm statistics ----
                mv2 = small.tile([128, 2, 2], F32)  # [:, t, (mean, E[x^2])]
                for t in range(2):
                    mv = small.tile([128, 2], F32, tag="bnmv")
                    nc.vector.bn_aggr(out=mv, in_=st[t])
                    nc.vector.tensor_copy(out=mv2[:, t, 0:1], in_=mv[:, 0:1])
                    nc.vector.tensor_tensor(
                        out=mv2[:, t, 1:2], in0=mv[:, 0:1], in1=mv[:, 0:1],
                        op=Alu.mult,
                    )
                    nc.vector.tensor_tensor(
                        out=mv2[:, t, 1:2], in0=mv2[:, t, 1:2], in1=mv[:, 1:2],
                        op=Alu.add,
                    )

                gps = s_ps.tile([128, 1024], F32, tag="sps", name="gnps")
                for t in range(2):
                    nc.tensor.matmul(
                        out=gps[:, 0:2], lhsT=GM[:, t], rhs=mv2[:, t],
                        start=(t == 0), stop=(t == 1),
                    )
                gsb = small.tile([128, 2], F32)
                nc.vector.tensor_copy(out=gsb, in_=gps[:, 0:2])
                # gstat rows 0..8: col0 = group mean, col1 = rsqrt(var+eps);
                # rows 8..128 stay zero for the padded broadcast matmul.
                gstat = small.tile([128, 2], F32)
                nc.vector.memset(gstat, 0.0)
                nc.vector.tensor_copy(out=gstat[:G, 0:1], in_=gsb[:G, 0:1])
                gvar = small.tile([G, 1], F32)
                nc.vector.tensor_tensor(
                    out=gvar, in0=gsb[:G, 0:1], in1=gsb[:G, 0:1], op=Alu.mult
                )
                nc.vector.tensor_tensor(
                    out=gvar, in0=gsb[:G, 1:2], in1=gvar, op=Alu.subtract
                )
                # rsqrt via exp(-0.5*ln(v+eps)): stays in the ln/exp table set
                nc.scalar.activation(out=gvar, in_=gvar, func=Ln, bias=EPS8)
                nc.scalar.activation(
                    out=gstat[:G, 1:2], in_=gvar, func=Exp, scale=-0.5
                )

                # broadcast group stats back to channels
                AB = []  # [t] -> [128, 2] (alpha, beta)
                for t in range(2):
                    bc = s_ps.tile([128, 1024], F32, tag="sps", name="bcst")
                    nc.tensor.matmul(out=bc[:, 0:2], lhsT=GMT[:, t], rhs=gstat)
                    bsb = small.tile([128, 2], F32, tag="bsb", name="bsb")
                    nc.vector.tensor_copy(out=bsb, in_=bc[:, 0:2])
                    ab = small.tile([128, 2], F32, tag=f"ab{t}", name=f"ab{t}")
                    # alpha = rstd * w
                    nc.vector.tensor_tensor(
                        out=ab[:, 0:1], in0=bsb[:, 1:2], in1=WN[:, t : t + 1],
                        op=Alu.mult,
                    )
                    # beta = b - mean * alpha
                    nc.vector.tensor_tensor(
                        out=ab[:, 1:2], in0=bsb[:, 0:1], in1=ab[:, 0:1],
                        op=Alu.mult,
                    )
                    nc.vector.tensor_tensor(
                        out=ab[:, 1:2], in0=BN[:, t : t + 1], in1=ab[:, 1:2],
                        op=Alu.subtract,
                    )
                    AB.append(ab)

                # ---- normalized queries + q projection ----
                for t in range(2):
                    nc.vector.tensor_scalar(
                        out=HQ[t], in0=XQ[t],
                        scalar1=AB[t][:, 0:1], scalar2=AB[t][:, 1:2],
                        op0=Alu.mult, op1=Alu.add,
                    )
                for t in range(2):
                    for half in range(2):
                        ps = s_ps.tile([128, 1024], F32, tag="sps", name="qps")
                        for sub in range(2):
                            nq = 2 * half + sub
                            for ct in range(2):
                                nc.tensor.matmul(
                                    out=ps[:, sub * 512 : (sub + 1) * 512],
                                    lhsT=WQ[:, ct, t * 128 : (t + 1) * 128],
                                    rhs=HQ[ct][:, nq * 512 : (nq + 1) * 512],
                                    start=(ct == 0), stop=(ct == 1),
                                )
                        nc.vector.tensor_scalar_add(
                            out=QT[t][:, half * 1024 : (half + 1) * 1024],
                            in0=ps, scalar1=BQ[:, t : t + 1],
                        )

                def kv_chunk(n):
                    # normalized activations for keys 512n..512n+512, then
                    # K (4 heads, zero-padded) and V^T (4 key tiles) for them
                    for t in range(2):
                        nc.vector.tensor_scalar(
                            out=H[t][:, n * 512 : (n + 1) * 512],
                            in0=X[t][:, n * 512 : (n + 1) * 512],
                            scalar1=AB[t][:, 0:1], scalar2=AB[t][:, 1:2],
                            op0=Alu.mult, op1=Alu.add,
                        )
                    for hp in range(2):
                        ps = s_ps.tile([128, 1024], F32, tag="sps", name="kps")
                        for h2 in range(2):
                            h = 2 * hp + h2
                            for ct in range(2):
                                nc.tensor.matmul(
                                    out=ps[:, h2 * 512 : (h2 + 1) * 512],
                                    lhsT=WKZ[:, ct, h],
                                    rhs=H[ct][:, n * 512 : (n + 1) * 512],
                                    start=(ct == 0), stop=(ct == 1),
                                )
                        for h2 in range(2):
                            h = 2 * hp + h2
                            nc.vector.tensor_scalar_add(
                                out=KZ[h][:, n * 512 : (n + 1) * 512],
                                in0=ps[:, h2 * 512 : (h2 + 1) * 512],
                                scalar1=BKZ[:, h : h + 1],
                            )
                    for jp in range(2):
                        ps = s_ps.tile([128, 1024], F32, tag="sps", name="vps")
                        for jo in range(2):
                            j = 4 * n + 2 * jp + jo
                            for ct in range(2):
                                nc.tensor.matmul(
                                    out=ps[:, jo * 512 : jo * 512 + NH * 65],
                                    lhsT=H[ct][:, j * 128 : (j + 1) * 128],
                                    rhs=WV[:, ct],
                                    start=(ct == 0), stop=(ct == 1),
                                )
                        for jo in range(2):
                            j = 4 * n + 2 * jp + jo
                            nc.vector.tensor_tensor(
                                out=V[:, j],
                                in0=ps[:, jo * 512 : jo * 512 + NH * 65],
                                in1=VB, op=Alu.add,
                            )

                kv_chunk(0)

                # ---- attention + projection ----
                for ic in range(NIC):
                    accs = [
                        acc_ps.tile([128, 512], F32, tag=f"acc{h}", name=f"acc{h}")
                        for h in range(NH)
                    ]
                    for n in range(NKC):
                        for j in range(4 * n, 4 * n + 4):
                            for sp in range(2):
                                S = s_ps.tile([128, 1024], F32, tag="sps", name="sps")
                                for h2 in range(2):
                                    nc.tensor.matmul(
                                        out=S[:, h2 * 512 : (h2 + 1) * 512],
                                        lhsT=KZ[2 * sp + h2][
                                            :, j * 128 : (j + 1) * 128
                                        ],
                                        rhs=QT[sp][:, ic * 512 : (ic + 1) * 512],
                                        start=True, stop=True,
                                    )
                                E = expp.tile([128, 1024], BF16, tag="exps", name="e")
                                nc.scalar.activation(out=E, in_=S, func=Exp, scale=SCALE)
                                for h2 in range(2):
                                    h = 2 * sp + h2
                                    nc.tensor.matmul(
                                        out=accs[h][0:65, :],
                                        lhsT=V[:, j, h * 65 : (h + 1) * 65],
                                        rhs=E[:, h2 * 512 : (h2 + 1) * 512],
                                        start=(j == 0), stop=(j == NJT - 1),
                                    )
                        if ic == 0 and n + 1 < NKC:
                            kv_chunk(n + 1)
                    # normalize by the denominator row (acc row 0 — the V^T
                    # tiles carry the ones column FIRST so everything stays at
                    # partition base 0 for DVE/gpsimd ops)
                    OH = []
                    for h in range(NH):
                        dc = small.tile([1, 512], F32, tag="dcp", name="dcp")
                        nc.vector.tensor_copy(out=dc, in_=accs[h][0:1, :])
                        r = small.tile([1, 512], F32, tag="recip", name="recip")
                        rs = small.tile([1, 512], F32, tag="rscr", name="rscr")
                        nc.vector.reciprocal_approx_accurate(r, dc, rs)
                        R = att.tile([65, 512], F32, tag="rbc", name="rbc")
                        nc.gpsimd.partition_broadcast(R, r)
                        oh = att.tile([65, 512], BF16, tag=f"oh{h}", name=f"oh{h}")
                        nc.vector.tensor_tensor(
                            out=oh, in0=accs[h][0:65, :], in1=R, op=Alu.mult,
                        )
                        OH.append(oh)
                    # proj + bias + residual (per-head K=65 matmuls; weight row
                    # 0 is zero, discarding the normalized-denominator row)
                    for mt in range(2):
                        pj = acc_ps.tile([128, 512], F32, tag=f"acc{mt}", name="pj")
                        for h in range(NH):
                            nc.tensor.matmul(
                                out=pj,
                                lhsT=WP[:, h, mt],
                                rhs=OH[h],
                                start=(h == 0), stop=(h == NH - 1),
                            )
                        ob = att.tile([128, 512], F32, tag="outsb", name="outsb")
                        nc.vector.tensor_scalar_add(
                            out=ob, in0=pj, scalar1=BP[:, mt : mt + 1]
                        )
                        nc.gpsimd.tensor_tensor(
                            out=ob, in0=ob,
                            in1=XQ[mt][:, ic * 512 : (ic + 1) * 512],
                            op=Alu.add,
                        )
                        nc.sync.dma_start(
                            out=out[
                                mt * 128 : (mt + 1) * 128,
                                ic * 512 : (ic + 1) * 512,
                            ],
                            in_=ob,
                        )
    if finalize:
        nc.finalize()
    return nc


def _prep_weights(norm_w, norm_b, qkv_w, qkv_b, proj_w, proj_b):
    """Host-side layout (pure reshapes/transposes + dtype casts of weights)."""
    import ml_dtypes

    f = np.float32
    cdt = ml_dtypes.bfloat16

    def ctile(v):  # (256,) -> (128, 2) per channel-tile columns
        return np.ascontiguousarray(np.asarray(v).reshape(2, 128).T, dtype=f)

    def ptile(m):  # (256, N) -> (128, 2, N)
        return np.ascontiguousarray(
            np.asarray(m).reshape(2, 128, -1).transpose(1, 0, 2), dtype=f
        )

    qkv_w = np.asarray(qkv_w)
    qkv_b = np.asarray(qkv_b)
    wqT = qkv_w[:C].T  # (256, 256)
    wkT = qkv_w[C : 2 * C].T  # (256, 256) key rows
    # per-head K weights, zero-padded so each head's output occupies the same
    # 64 partition rows as its q in the packed 2-head Q tile
    wkzT = np.zeros((C, NH, 128), dtype=f)
    bkz = np.zeros((128, NH), dtype=f)
    for h in range(NH):
        off = 64 * (h % 2)
        wkzT[:, h, off : off + 64] = wkT[:, h * 64 : (h + 1) * 64]
        bkz[off : off + 64, h] = qkv_b[C + h * 64 : C + (h + 1) * 64]
    wvm = qkv_w[2 * C :]  # (256, 256)
    wvT = np.zeros((C, NH * 65), dtype=f)
    vb = np.zeros((128, NH * 65), dtype=f)
    for h in range(NH):
        wvT[:, h * 65 + 1 : h * 65 + 65] = wvm[h * 64 : (h + 1) * 64].T
        vb[:, h * 65 + 1 : h * 65 + 65] = qkv_b[
            2 * C + h * 64 : 2 * C + (h + 1) * 64
        ][None, :]
        vb[:, h * 65] = 1.0  # leading ones column -> denominator at psum row 0
    # zero-padded group masks (value 1/32 for group-mean aggregation; one-hot
    # transpose for the broadcast back to channels)
    gm = np.zeros((C, 128), dtype=f)
    for c in range(C):
        gm[c, c // 32] = 1.0 / 32.0
    # gmaskT param layout [p, t, 128]: partition p = group index (only 0..8
    # nonzero), free = channel within tile t
    gmaskT = np.zeros((128, 2, 128), dtype=f)
    for c in range(C):
        gmaskT[c // 32, c // 128, c % 128] = 1.0

    def wph(pw):  # (256 out, 256 in) -> [65, NH, 2, 128]; row 0 stays zero
        w = np.zeros((65, NH, 2, 128), dtype=f)
        for h in range(NH):
            for mt in range(2):
                w[1:, h, mt, :] = pw[
                    mt * 128 : (mt + 1) * 128, h * 64 : (h + 1) * 64
                ].T
        return w
    return dict(
        wn2=ctile(norm_w),
        bn2=ctile(norm_b),
        wq=ptile(wqT).astype(cdt),
        bq2=np.ascontiguousarray(qkv_b[:C].reshape(2, 128).T, dtype=f),
        wkz=ptile(wkzT.reshape(C, NH * 128))
        .reshape(128, 2, NH, 128)
        .astype(cdt),
        bkz=bkz,
        wv=ptile(wvT).astype(cdt),
        vb=vb,
        wproj=wph(np.asarray(proj_w)).astype(cdt),
        bproj2=ctile(proj_b),
        gmask=ptile(gm),
        gmaskT=gmaskT,
    )


_NC_CACHE = {}
_RUN_OPTS = {}  # extra kwargs for run_bass_kernel_spmd (test harness sets trace)
LAST_RESULT = None


def _get_nc():
    if "nc" not in _NC_CACHE:
        _NC_CACHE["nc"] = build()
    return _NC_CACHE["nc"]


def kernel(x, norm_w, norm_b, qkv_w, qkv_b, proj_w, proj_b, **_):
    nc = _get_nc()
    w = _prep_weights(norm_w, norm_b, qkv_w, qkv_b, proj_w, proj_b)
    x = np.asarray(x, dtype=np.float32)
    Bv, Cv, Hv, Wv = x.shape
    xf = x.reshape(Bv, Cv, Hv * Wv)
    in_maps = []
    for j in range(8):
        b, qh = j // 2, j % 2
        m = dict(w)
        m["x"] = np.ascontiguousarray(xf[b])
        m["xq"] = np.ascontiguousarray(xf[b][:, qh * Q : (qh + 1) * Q])
        in_maps.append(m)
    res = run_bass_kernel_spmd(nc, in_maps, core_ids=list(range(8)), **_RUN_OPTS)
    global LAST_RESULT
    LAST_RESULT = res
    outf = np.empty((Bv, Cv, Hv * Wv), dtype=np.float32)
    for j in range(8):
        b, qh = j // 2, j % 2
        outf[b][:, qh * Q : (qh + 1) * Q] = res.results[j]["out"]
    return outf.reshape(Bv, Cv, Hv, Wv)


# revision 10
# speedup vs baseline: 1.0486x; 1.0486x over previous
"""AttentionBlock (GroupNorm -> qkv 1x1 -> 4-head attention over 4096 tokens
-> proj 1x1 -> residual) distributed over 8 TRN2 NeuronCores.

Sharding: zero-communication query sharding. Core j handles batch b = j//2 and
query half qh = j%2 (2048 of the 4096 spatial positions). Each core loads the
full x[b] (256, 4096), computes GroupNorm + K/V over all keys, Q only for its
2048 queries, and writes its (256, 2048) output slice.

Structure (v2, tuned for Act-engine saturation):
  - scores are computed transposed ([keys, queries] via lhsT=k, rhs=q) so the
    exp() output feeds the AV matmul as the *moving* operand: AV uses V^T as
    the stationary tensor (lhsT = [keys, d+1] per head, with a ones column
    appended so the softmax denominator accumulates as psum row 64) and
    streams 512-query tiles of exp(scores). This removes the per-tile
    LDWEIGHTS of exp(scores) and the output transposes of the previous
    oT-form, and leaves attention output directly in [channels, queries].
  - softmax max-subtraction is skipped (logits are O(4) std-normal, far from
    fp32/bf16 exp overflow); per-query normalization is applied after AV via
    an approx reciprocal of the denominator row, partition-broadcast to the
    64 head channels.
  - one exp() per [128 keys, 2 heads x 512 queries] psum tile; the j-loop
    covers 4 heads flat so PSUM is exactly 8 banks: 2x[128,1024] f32 score
    tiles (double buffered) + 4x[128,512] AV accumulators (the proj psum
    recycles accumulator banks).
  - the prologue is pipelined: x is DMA'd in chunks with bn_stats emitted per
    chunk, and K/V production for key-chunk n+1 is interleaved into the main
    loop of the first query chunk, so the Act engine starts exp'ing as early
    as possible.
  - matmuls run in bf16 (full 128x128 PE, K=128 zero-padded per-head K
    tensors); GroupNorm stats, softmax accumulation/normalization and the
    residual stay fp32.
"""

import numpy as np

import concourse.bass as bass
import concourse.tile as tile
from concourse import bacc, mybir
from concourse.bass_utils import run_bass_kernel_spmd

C = 256
HW = 4096
NH = 4
D = 64  # head dim
G = 8  # groups
EPS = 1e-5
SCALE = D**-0.5
Q = HW // 2  # queries per core
NJT = HW // 128  # 32 key tiles
NKC = 8  # key chunks (512 keys each) for K/V production
NIC = Q // 512  # 4 query chunks of 512

F32 = mybir.dt.float32
BF16 = mybir.dt.bfloat16


def build(finalize=True):
    nc = bacc.Bacc("TRN2", target_bir_lowering=False, debug=False, num_devices=8)

    x = nc.declare_dram_parameter("x", [C, HW], F32, isOutput=False)
    xq = nc.declare_dram_parameter("xq", [C, Q], F32, isOutput=False)
    wn2 = nc.declare_dram_parameter("wn2", [128, 2], F32, isOutput=False)
    bn2 = nc.declare_dram_parameter("bn2", [128, 2], F32, isOutput=False)
    wq = nc.declare_dram_parameter("wq", [128, 2, C], BF16, isOutput=False)
    bq2 = nc.declare_dram_parameter("bq2", [128, 2], F32, isOutput=False)
    wkz = nc.declare_dram_parameter("wkz", [128, 2, NH, 128], BF16, isOutput=False)
    bkz = nc.declare_dram_parameter("bkz", [128, NH], F32, isOutput=False)
    wv = nc.declare_dram_parameter("wv", [128, 2, NH * 65], BF16, isOutput=False)
    vb = nc.declare_dram_parameter("vb", [128, NH * 65], F32, isOutput=False)
    wproj = nc.declare_dram_parameter("wproj", [65, NH, 2, 128], BF16, isOutput=False)
    bproj2 = nc.declare_dram_parameter("bproj2", [128, 2], F32, isOutput=False)
    gmask = nc.declare_dram_parameter("gmask", [128, 2, 128], F32, isOutput=False)
    gmaskT = nc.declare_dram_parameter("gmaskT", [128, 2, 128], F32, isOutput=False)
    out = nc.declare_dram_parameter("out", [C, Q], F32, isOutput=True)

    Exp = mybir.ActivationFunctionType.Exp
    Ln = mybir.ActivationFunctionType.Ln
    Alu = mybir.AluOpType

    with tile.TileContext(nc) as tc:
        with (
            tc.tile_pool(name="keep", bufs=1) as keep,
            tc.tile_pool(name="consts", bufs=1) as consts,
            tc.tile_pool(name="small", bufs=4) as small,
            tc.tile_pool(name="s_ps", bufs=2, space="PSUM") as s_ps,
            tc.tile_pool(name="acc_ps", bufs=1, space="PSUM") as acc_ps,
            tc.tile_pool(name="exps", bufs=4) as expp,
            tc.tile_pool(name="att", bufs=2) as att,
        ):
            XQ = [
                keep.tile([128, Q], F32, tag=f"XQ{t}", name=f"XQ{t}")
                for t in range(2)
            ]
            for t in range(2):
                nc.sync.dma_start(out=XQ[t], in_=xq[t * 128 : (t + 1) * 128, :])

            # persistent attention operands
            KZ = [
                keep.tile([128, HW], BF16, tag=f"KZ{h}", name=f"KZ{h}")
                for h in range(NH)
            ]
            QT = [
                keep.tile([128, Q], BF16, tag=f"Q{t}", name=f"Q{t}")
                for t in range(2)
            ]
            # V^T with a leading ones column per head: [keys, (head, 1+d)]
            V = keep.tile([128, NJT, NH * 65], BF16)

            with tc.tile_pool(name="xh", bufs=1) as xh:
                X = [
                    xh.tile([128, HW], F32, tag=f"X{t}", name=f"X{t}")
                    for t in range(2)
                ]
                H = [
                    xh.tile([128, HW], BF16, tag=f"H{t}", name=f"H{t}")
                    for t in range(2)
                ]
                HQ = [
                    xh.tile([128, Q], BF16, tag=f"HQ{t}", name=f"HQ{t}")
                    for t in range(2)
                ]

                # ---- x DMA in chunks, bn_stats per chunk ----
                st = [
                    small.tile([128, 8, 6], F32, tag=f"bnst{t}", name=f"bnst{t}")
                    for t in range(2)
                ]
                for ch in range(4):
                    for t in range(2):
                        nc.sync.dma_start(
                            out=X[t][:, ch * 1024 : (ch + 1) * 1024],
                            in_=x[t * 128 : (t + 1) * 128, ch * 1024 : (ch + 1) * 1024],
                        )
                        xr = X[t].rearrange("p (n f) -> p n f", f=512)
                        for s in (2 * ch, 2 * ch + 1):
                            nc.vector.bn_stats(out=st[t][:, s], in_=xr[:, s])

                # ---- weights (after the x chunks in queue order) ----
                GM = consts.tile([128, 2, 128], F32)
                nc.sync.dma_start(out=GM, in_=gmask[:])
                GMT = consts.tile([128, 2, 128], F32)
                nc.sync.dma_start(out=GMT, in_=gmaskT[:])
                WN = consts.tile([128, 2], F32)
                nc.sync.dma_start(out=WN, in_=wn2[:])
                BN = consts.tile([128, 2], F32)
                nc.sync.dma_start(out=BN, in_=bn2[:])
                WQ = consts.tile([128, 2, C], BF16)
                nc.sync.dma_start(out=WQ, in_=wq[:])
                BQ = consts.tile([128, 2], F32)
                nc.sync.dma_start(out=BQ, in_=bq2[:])
                WKZ = consts.tile([128, 2, NH, 128], BF16)
                nc.sync.dma_start(out=WKZ, in_=wkz[:])
                BKZ = consts.tile([128, NH], F32)
                nc.sync.dma_start(out=BKZ, in_=bkz[:])
                WV = consts.tile([128, 2, NH * 65], BF16)
                nc.sync.dma_start(out=WV, in_=wv[:])
                VB = consts.tile([128, NH * 65], F32)
                nc.sync.dma_start(out=VB, in_=vb[:])
                WP = consts.tile([65, NH, 2, 128], BF16)
                nc.sync.dma_start(out=WP, in_=wproj[:])
                BP = consts.tile([128, 2], F32)
                nc.sync.dma_start(out=BP, in_=bproj2[:])
                EPS8 = consts.tile([G, 1], F32)
                nc.vector.memset(EPS8, EPS)

                # ---- GroupNorm statistics ----
                mv2 = small.tile([128, 2, 2], F32)  # [:, t, (mean, E[x^2])]
                for t in range(2):
                    mv = small.tile([128, 2], F32, tag="bnmv")
                    nc.vector.bn_aggr(out=mv, in_=st[t])
                    nc.vector.tensor_copy(out=mv2[:, t, 0:1], in_=mv[:, 0:1])
                    nc.vector.tensor_tensor(
                        out=mv2[:, t, 1:2], in0=mv[:, 0:1], in1=mv[:, 0:1],
                        op=Alu.mult,
                    )
                    nc.vector.tensor_tensor(
                        out=mv2[:, t, 1:2], in0=mv2[:, t, 1:2], in1=mv[:, 1:2],
                        op=Alu.add,
                    )

                gps = s_ps.tile([128, 1024], F32, tag="sps", name="gnps")
                for t in range(2):
                    nc.tensor.matmul(
                        out=gps[:, 0:2], lhsT=GM[:, t], rhs=mv2[:, t],
                        start=(t == 0), stop=(t == 1),
                    )
                gsb = small.tile([128, 2], F32)
                nc.vector.tensor_copy(out=gsb, in_=gps[:, 0:2])
                # gstat rows 0..8: col0 = group mean, col1 = rsqrt(var+eps);
                # rows 8..128 stay zero for the padded broadcast matmul.
                gstat = small.tile([128, 2], F32)
                nc.vector.memset(gstat, 0.0)
                nc.vector.tensor_copy(out=gstat[:G, 0:1], in_=gsb[:G, 0:1])
                gvar = small.tile([G, 1], F32)
                nc.vector.tensor_tensor(
                    out=gvar, in0=gsb[:G, 0:1], in1=gsb[:G, 0:1], op=Alu.mult
                )
                nc.vector.tensor_tensor(
                    out=gvar, in0=gsb[:G, 1:2], in1=gvar, op=Alu.subtract
                )
                # rsqrt via exp(-0.5*ln(v+eps)): stays in the ln/exp table set
                nc.scalar.activation(out=gvar, in_=gvar, func=Ln, bias=EPS8)
                nc.scalar.activation(
                    out=gstat[:G, 1:2], in_=gvar, func=Exp, scale=-0.5
                )

                # broadcast group stats back to channels
                AB = []  # [t] -> [128, 2] (alpha, beta)
                for t in range(2):
                    bc = s_ps.tile([128, 1024], F32, tag="sps", name="bcst")
                    nc.tensor.matmul(out=bc[:, 0:2], lhsT=GMT[:, t], rhs=gstat)
                    bsb = small.tile([128, 2], F32, tag="bsb", name="bsb")
                    nc.vector.tensor_copy(out=bsb, in_=bc[:, 0:2])
                    ab = small.tile([128, 2], F32, tag=f"ab{t}", name=f"ab{t}")
                    # alpha = rstd * w
                    nc.vector.tensor_tensor(
                        out=ab[:, 0:1], in0=bsb[:, 1:2], in1=WN[:, t : t + 1],
                        op=Alu.mult,
                    )
                    # beta = b - mean * alpha
                    nc.vector.tensor_tensor(
                        out=ab[:, 1:2], in0=bsb[:, 0:1], in1=ab[:, 0:1],
                        op=Alu.mult,
                    )
                    nc.vector.tensor_tensor(
                        out=ab[:, 1:2], in0=BN[:, t : t + 1], in1=ab[:, 1:2],
                        op=Alu.subtract,
                    )
                    AB.append(ab)

                # ---- normalized activations: queries on DVE, keys on gpsimd
                for t in range(2):
                    nc.vector.tensor_scalar(
                        out=HQ[t], in0=XQ[t],
                        scalar1=AB[t][:, 0:1], scalar2=AB[t][:, 1:2],
                        op0=Alu.mult, op1=Alu.add,
                    )
                    nc.gpsimd.tensor_scalar(
                        out=H[t], in0=X[t],
                        scalar1=AB[t][:, 0:1], scalar2=AB[t][:, 1:2],
                        op0=Alu.mult, op1=Alu.add,
                    )

                # ---- q projection ----
                for t in range(2):
                    for half in range(2):
                        ps = s_ps.tile([128, 1024], F32, tag="sps", name="qps")
                        for sub in range(2):
                            nq = 2 * half + sub
                            for ct in range(2):
                                nc.tensor.matmul(
                                    out=ps[:, sub * 512 : (sub + 1) * 512],
                                    lhsT=WQ[:, ct, t * 128 : (t + 1) * 128],
                                    rhs=HQ[ct][:, nq * 512 : (nq + 1) * 512],
                                    start=(ct == 0), stop=(ct == 1),
                                )
                        nc.vector.tensor_scalar_add(
                            out=QT[t][:, half * 1024 : (half + 1) * 1024],
                            in0=ps, scalar1=BQ[:, t : t + 1],
                        )

                def kv_piece(n, p):
                    # piece p (0,1) = K head-pair p; (2,3) = V key-tile pair
                    if p < 2:
                        hp = p
                        ps = s_ps.tile([128, 1024], F32, tag="sps", name="kps")
                        for h2 in range(2):
                            h = 2 * hp + h2
                            for ct in range(2):
                                nc.tensor.matmul(
                                    out=ps[:, h2 * 512 : (h2 + 1) * 512],
                                    lhsT=WKZ[:, ct, h],
                                    rhs=H[ct][:, n * 512 : (n + 1) * 512],
                                    start=(ct == 0), stop=(ct == 1),
                                )
                        for h2 in range(2):
                            h = 2 * hp + h2
                            nc.vector.tensor_scalar_add(
                                out=KZ[h][:, n * 512 : (n + 1) * 512],
                                in0=ps[:, h2 * 512 : (h2 + 1) * 512],
                                scalar1=BKZ[:, h : h + 1],
                            )
                    else:
                        jp = p - 2
                        ps = s_ps.tile([128, 1024], F32, tag="sps", name="vps")
                        for jo in range(2):
                            j = 4 * n + 2 * jp + jo
                            for ct in range(2):
                                nc.tensor.matmul(
                                    out=ps[:, jo * 512 : jo * 512 + NH * 65],
                                    lhsT=H[ct][:, j * 128 : (j + 1) * 128],
                                    rhs=WV[:, ct],
                                    start=(ct == 0), stop=(ct == 1),
                                )
                        for jo in range(2):
                            j = 4 * n + 2 * jp + jo
                            nc.vector.tensor_tensor(
                                out=V[:, j],
                                in0=ps[:, jo * 512 : jo * 512 + NH * 65],
                                in1=VB, op=Alu.add,
                            )

                for n in range(2):
                    for p in range(4):
                        kv_piece(n, p)

                # ---- attention + projection ----
                def normalize(accs):
                    # per-head o tiles, normalized by the denominator row
                    # (acc row 0; everything stays at partition base 0)
                    OH = []
                    for h in range(NH):
                        dc = small.tile([1, 512], F32, tag="dcp", name="dcp")
                        nc.vector.tensor_copy(out=dc, in_=accs[h][0:1, :])
                        r = small.tile([1, 512], F32, tag="recip", name="recip")
                        rs = small.tile([1, 512], F32, tag="rscr", name="rscr")
                        nc.vector.reciprocal_approx_accurate(r, dc, rs)
                        R = att.tile([65, 512], F32, tag="rbc", name="rbc")
                        nc.gpsimd.partition_broadcast(R, r)
                        oh = att.tile([65, 512], BF16, tag=f"oh{h}", name=f"oh{h}")
                        nc.vector.tensor_tensor(
                            out=oh, in0=accs[h][0:65, :], in1=R, op=Alu.mult,
                        )
                        OH.append(oh)
                    return OH

                def project(OH, ic):
                    # proj + bias + residual (per-head K=65 matmuls; weight
                    # row 0 is zero, discarding the denominator row)
                    for mt in range(2):
                        pjt = s_ps.tile([128, 1024], F32, tag="sps", name="pj")
                        pj = pjt[:, 0:512]
                        for h in range(NH):
                            nc.tensor.matmul(
                                out=pj,
                                lhsT=WP[:, h, mt],
                                rhs=OH[h],
                                start=(h == 0), stop=(h == NH - 1),
                            )
                        ob = att.tile([128, 512], F32, tag="outsb", name="outsb")
                        nc.vector.tensor_scalar_add(
                            out=ob, in0=pj, scalar1=BP[:, mt : mt + 1]
                        )
                        nc.gpsimd.tensor_tensor(
                            out=ob, in0=ob,
                            in1=XQ[mt][:, ic * 512 : (ic + 1) * 512],
                            op=Alu.add,
                        )
                        nc.sync.dma_start(
                            out=out[
                                mt * 128 : (mt + 1) * 128,
                                ic * 512 : (ic + 1) * 512,
                            ],
                            in_=ob,
                        )

                pending = None  # (OH, ic) awaiting projection
                for ic in range(NIC):
                    accs = [
                        acc_ps.tile([128, 512], F32, tag=f"acc{h}", name=f"acc{h}")
                        for h in range(NH)
                    ]
                    for n in range(NKC):
                        for jj in range(4):
                            j = 4 * n + jj
                            for sp in range(2):
                                S = s_ps.tile([128, 1024], F32, tag="sps", name="sps")
                                for h2 in range(2):
                                    nc.tensor.matmul(
                                        out=S[:, h2 * 512 : (h2 + 1) * 512],
                                        lhsT=KZ[2 * sp + h2][
                                            :, j * 128 : (j + 1) * 128
                                        ],
                                        rhs=QT[sp][:, ic * 512 : (ic + 1) * 512],
                                        start=True, stop=True,
                                    )
                                E = expp.tile([128, 1024], BF16, tag="exps", name="e")
                                nc.scalar.activation(out=E, in_=S, func=Exp, scale=SCALE)
                                for h2 in range(2):
                                    h = 2 * sp + h2
                                    nc.tensor.matmul(
                                        out=accs[h][0:65, :],
                                        lhsT=V[:, j, h * 65 : (h + 1) * 65],
                                        rhs=E[:, h2 * 512 : (h2 + 1) * 512],
                                        start=(j == 0), stop=(j == NJT - 1),
                                    )
                            # interleave K/V production two chunks ahead
                            if ic == 0 and n + 2 < NKC:
                                kv_piece(n + 2, jj)
                        if n == 0 and pending is not None:
                            project(*pending)
                            pending = None
                    pending = (normalize(accs), ic)
                project(*pending)
    if finalize:
        nc.finalize()
    return nc


def _prep_weights(norm_w, norm_b, qkv_w, qkv_b, proj_w, proj_b):
    """Host-side layout (pure reshapes/transposes + dtype casts of weights)."""
    import ml_dtypes

    f = np.float32
    cdt = ml_dtypes.bfloat16

    def ctile(v):  # (256,) -> (128, 2) per channel-tile columns
        return np.ascontiguousarray(np.asarray(v).reshape(2, 128).T, dtype=f)

    def ptile(m):  # (256, N) -> (128, 2, N)
        return np.ascontiguousarray(
            np.asarray(m).reshape(2, 128, -1).transpose(1, 0, 2), dtype=f
        )

    qkv_w = np.asarray(qkv_w)
    qkv_b = np.asarray(qkv_b)
    wqT = qkv_w[:C].T  # (256, 256)
    wkT = qkv_w[C : 2 * C].T  # (256, 256) key rows
    # per-head K weights, zero-padded so each head's output occupies the same
    # 64 partition rows as its q in the packed 2-head Q tile
    wkzT = np.zeros((C, NH, 128), dtype=f)
    bkz = np.zeros((128, NH), dtype=f)
    for h in range(NH):
        off = 64 * (h % 2)
        wkzT[:, h, off : off + 64] = wkT[:, h * 64 : (h + 1) * 64]
        bkz[off : off + 64, h] = qkv_b[C + h * 64 : C + (h + 1) * 64]
    wvm = qkv_w[2 * C :]  # (256, 256)
    wvT = np.zeros((C, NH * 65), dtype=f)
    vb = np.zeros((128, NH * 65), dtype=f)
    for h in range(NH):
        wvT[:, h * 65 + 1 : h * 65 + 65] = wvm[h * 64 : (h + 1) * 64].T
        vb[:, h * 65 + 1 : h * 65 + 65] = qkv_b[
            2 * C + h * 64 : 2 * C + (h + 1) * 64
        ][None, :]
        vb[:, h * 65] = 1.0  # leading ones column -> denominator at psum row 0
    # zero-padded group masks (value 1/32 for group-mean aggregation; one-hot
    # transpose for the broadcast back to channels)
    gm = np.zeros((C, 128), dtype=f)
    for c in range(C):
        gm[c, c // 32] = 1.0 / 32.0
    # gmaskT param layout [p, t, 128]: partition p = group index (only 0..8
    # nonzero), free = channel within tile t
    gmaskT = np.zeros((128, 2, 128), dtype=f)
    for c in range(C):
        gmaskT[c // 32, c // 128, c % 128] = 1.0

    def wph(pw):  # (256 out, 256 in) -> [65, NH, 2, 128]; row 0 stays zero
        w = np.zeros((65, NH, 2, 128), dtype=f)
        for h in range(NH):
            for mt in range(2):
                w[1:, h, mt, :] = pw[
                    mt * 128 : (mt + 1) * 128, h * 64 : (h + 1) * 64
                ].T
        return w
    return dict(
        wn2=ctile(norm_w),
        bn2=ctile(norm_b),
        wq=ptile(wqT).astype(cdt),
        bq2=np.ascontiguousarray(qkv_b[:C].reshape(2, 128).T, dtype=f),
        wkz=ptile(wkzT.reshape(C, NH * 128))
        .reshape(128, 2, NH, 128)
        .astype(cdt),
        bkz=bkz,
        wv=ptile(wvT).astype(cdt),
        vb=vb,
        wproj=wph(np.asarray(proj_w)).astype(cdt),
        bproj2=ctile(proj_b),
        gmask=ptile(gm),
        gmaskT=gmaskT,
    )


_NC_CACHE = {}
_RUN_OPTS = {}  # extra kwargs for run_bass_kernel_spmd (test harness sets trace)
LAST_RESULT = None


def _get_nc():
    if "nc" not in _NC_CACHE:
        _NC_CACHE["nc"] = build()
    return _NC_CACHE["nc"]


def kernel(x, norm_w, norm_b, qkv_w, qkv_b, proj_w, proj_b, **_):
    nc = _get_nc()
    w = _prep_weights(norm_w, norm_b, qkv_w, qkv_b, proj_w, proj_b)
    x = np.asarray(x, dtype=np.float32)
    Bv, Cv, Hv, Wv = x.shape
    xf = x.reshape(Bv, Cv, Hv * Wv)
    in_maps = []
    for j in range(8):
        b, qh = j // 2, j % 2
        m = dict(w)
        m["x"] = np.ascontiguousarray(xf[b])
        m["xq"] = np.ascontiguousarray(xf[b][:, qh * Q : (qh + 1) * Q])
        in_maps.append(m)
    res = run_bass_kernel_spmd(nc, in_maps, core_ids=list(range(8)), **_RUN_OPTS)
    global LAST_RESULT
    LAST_RESULT = res
    outf = np.empty((Bv, Cv, Hv * Wv), dtype=np.float32)
    for j in range(8):
        b, qh = j // 2, j % 2
        outf[b][:, qh * Q : (qh + 1) * Q] = res.results[j]["out"]
    return outf.reshape(Bv, Cv, Hv, Wv)


# revision 11
# speedup vs baseline: 1.1704x; 1.1161x over previous
"""AttentionBlock (GroupNorm -> qkv 1x1 -> 4-head attention over 4096 tokens
-> proj 1x1 -> residual) distributed over 8 TRN2 NeuronCores.

Sharding: zero-communication query sharding. Core j handles batch b = j//2 and
query half qh = j%2 (2048 of the 4096 spatial positions). Each core loads the
full x[b] (256, 4096), computes GroupNorm + K/V over all keys, Q only for its
2048 queries, and writes its (256, 2048) output slice.

Structure (v2, tuned for Act-engine saturation):
  - scores are computed transposed ([keys, queries] via lhsT=k, rhs=q) so the
    exp() output feeds the AV matmul as the *moving* operand: AV uses V^T as
    the stationary tensor (lhsT = [keys, d+1] per head, with a ones column
    appended so the softmax denominator accumulates as psum row 64) and
    streams 512-query tiles of exp(scores). This removes the per-tile
    LDWEIGHTS of exp(scores) and the output transposes of the previous
    oT-form, and leaves attention output directly in [channels, queries].
  - softmax max-subtraction is skipped (logits are O(4) std-normal, far from
    fp32/bf16 exp overflow); per-query normalization is applied after AV via
    an approx reciprocal of the denominator row, partition-broadcast to the
    64 head channels.
  - one exp() per [128 keys, 2 heads x 512 queries] psum tile; the j-loop
    covers 4 heads flat so PSUM is exactly 8 banks: 2x[128,1024] f32 score
    tiles (double buffered) + 4x[128,512] AV accumulators (the proj psum
    recycles accumulator banks).
  - the prologue is pipelined: x is DMA'd in chunks with bn_stats emitted per
    chunk, and K/V production for key-chunk n+1 is interleaved into the main
    loop of the first query chunk, so the Act engine starts exp'ing as early
    as possible.
  - matmuls run in bf16 (full 128x128 PE, K=128 zero-padded per-head K
    tensors); GroupNorm stats, softmax accumulation/normalization and the
    residual stay fp32.
"""

import numpy as np

import concourse.bass as bass
import concourse.tile as tile
from concourse import bacc, mybir
from concourse.bass_utils import run_bass_kernel_spmd

C = 256
HW = 4096
NH = 4
D = 64  # head dim
G = 8  # groups
EPS = 1e-5
SCALE = D**-0.5
Q = HW // 2  # queries per core
NJT = HW // 128  # 32 key tiles
NKC = 8  # key chunks (512 keys each) for K/V production
NIC = Q // 512  # 4 query chunks of 512

F32 = mybir.dt.float32
BF16 = mybir.dt.bfloat16


def build(finalize=True):
    nc = bacc.Bacc("TRN2", target_bir_lowering=False, debug=False, num_devices=8)

    x = nc.declare_dram_parameter("x", [C, HW], F32, isOutput=False)
    xq = nc.declare_dram_parameter("xq", [C, Q], F32, isOutput=False)
    wn2 = nc.declare_dram_parameter("wn2", [128, 2], F32, isOutput=False)
    bn2 = nc.declare_dram_parameter("bn2", [128, 2], F32, isOutput=False)
    wq = nc.declare_dram_parameter("wq", [128, 2, C], BF16, isOutput=False)
    bq2 = nc.declare_dram_parameter("bq2", [128, 2], F32, isOutput=False)
    wkz = nc.declare_dram_parameter("wkz", [128, 2, NH, 128], BF16, isOutput=False)
    bkz = nc.declare_dram_parameter("bkz", [128, NH], F32, isOutput=False)
    wv = nc.declare_dram_parameter("wv", [128, 2, NH * 65], BF16, isOutput=False)
    vb = nc.declare_dram_parameter("vb", [128, NH * 65], F32, isOutput=False)
    wproj = nc.declare_dram_parameter("wproj", [65, NH, 2, 128], BF16, isOutput=False)
    bproj2 = nc.declare_dram_parameter("bproj2", [128, 2], F32, isOutput=False)
    gmask = nc.declare_dram_parameter("gmask", [128, 2, 128], F32, isOutput=False)
    gmaskT = nc.declare_dram_parameter("gmaskT", [128, 2, 128], F32, isOutput=False)
    out = nc.declare_dram_parameter("out", [C, Q], F32, isOutput=True)

    Exp = mybir.ActivationFunctionType.Exp
    Ln = mybir.ActivationFunctionType.Ln
    Alu = mybir.AluOpType
    AVLAG = 6  # AV matmuls trail QK/exp by this many S tiles (2 per key tile)

    with tile.TileContext(nc) as tc:
        with (
            tc.tile_pool(name="keep", bufs=1) as keep,
            tc.tile_pool(name="consts", bufs=1) as consts,
            tc.tile_pool(name="small", bufs=4) as small,
            tc.tile_pool(name="s_ps", bufs=2, space="PSUM") as s_ps,
            tc.tile_pool(name="acc_ps", bufs=1, space="PSUM") as acc_ps,
            tc.tile_pool(name="exps", bufs=AVLAG + 2) as expp,
            tc.tile_pool(name="att", bufs=2) as att,
        ):
            # persistent attention operands
            KZ = [
                keep.tile([128, HW], BF16, tag=f"KZ{h}", name=f"KZ{h}")
                for h in range(NH)
            ]
            QT = [
                keep.tile([128, Q], BF16, tag=f"Q{t}", name=f"Q{t}")
                for t in range(2)
            ]
            # V^T with a leading ones column per head: [keys, (head, 1+d)]
            V = keep.tile([128, NJT, NH * 65], BF16)
            XQ = [
                keep.tile([128, Q], F32, tag=f"XQ{t}", name=f"XQ{t}")
                for t in range(2)
            ]

            with tc.tile_pool(name="xh", bufs=1) as xh:
                X = [
                    xh.tile([128, HW], F32, tag=f"X{t}", name=f"X{t}")
                    for t in range(2)
                ]
                H = [
                    xh.tile([128, HW], BF16, tag=f"H{t}", name=f"H{t}")
                    for t in range(2)
                ]
                HQ = [
                    xh.tile([128, Q], BF16, tag=f"HQ{t}", name=f"HQ{t}")
                    for t in range(2)
                ]

                # ---- x DMA in chunks, bn_stats per chunk ----
                st = [
                    small.tile([128, 8, 6], F32, tag=f"bnst{t}", name=f"bnst{t}")
                    for t in range(2)
                ]
                for ch in range(4):
                    for t in range(2):
                        nc.sync.dma_start(
                            out=X[t][:, ch * 1024 : (ch + 1) * 1024],
                            in_=x[t * 128 : (t + 1) * 128, ch * 1024 : (ch + 1) * 1024],
                        )
                        xr = X[t].rearrange("p (n f) -> p n f", f=512)
                        for s in (2 * ch, 2 * ch + 1):
                            nc.vector.bn_stats(out=st[t][:, s], in_=xr[:, s])
                for t in range(2):
                    nc.sync.dma_start(out=XQ[t], in_=xq[t * 128 : (t + 1) * 128, :])

                # ---- weights (after the x chunks in queue order) ----
                GM = consts.tile([128, 2, 128], F32)
                nc.sync.dma_start(out=GM, in_=gmask[:])
                GMT = consts.tile([128, 2, 128], F32)
                nc.sync.dma_start(out=GMT, in_=gmaskT[:])
                WN = consts.tile([128, 2], F32)
                nc.sync.dma_start(out=WN, in_=wn2[:])
                BN = consts.tile([128, 2], F32)
                nc.sync.dma_start(out=BN, in_=bn2[:])
                WQ = consts.tile([128, 2, C], BF16)
                nc.sync.dma_start(out=WQ, in_=wq[:])
                BQ = consts.tile([128, 2], F32)
                nc.sync.dma_start(out=BQ, in_=bq2[:])
                WKZ = consts.tile([128, 2, NH, 128], BF16)
                nc.sync.dma_start(out=WKZ, in_=wkz[:])
                BKZ = consts.tile([128, NH], F32)
                nc.sync.dma_start(out=BKZ, in_=bkz[:])
                WV = consts.tile([128, 2, NH * 65], BF16)
                nc.sync.dma_start(out=WV, in_=wv[:])
                VB = consts.tile([128, NH * 65], F32)
                nc.sync.dma_start(out=VB, in_=vb[:])
                WP = consts.tile([65, NH, 2, 128], BF16)
                nc.sync.dma_start(out=WP, in_=wproj[:])
                BP = consts.tile([128, 2], F32)
                nc.sync.dma_start(out=BP, in_=bproj2[:])
                EPS8 = consts.tile([G, 1], F32)
                nc.vector.memset(EPS8, EPS)

                # ---- GroupNorm statistics ----
                mv2 = small.tile([128, 2, 2], F32)  # [:, t, (mean, E[x^2])]
                for t in range(2):
                    mv = small.tile([128, 2], F32, tag="bnmv")
                    nc.vector.bn_aggr(out=mv, in_=st[t])
                    nc.vector.tensor_copy(out=mv2[:, t, 0:1], in_=mv[:, 0:1])
                    nc.vector.tensor_tensor(
                        out=mv2[:, t, 1:2], in0=mv[:, 0:1], in1=mv[:, 0:1],
                        op=Alu.mult,
                    )
                    nc.vector.tensor_tensor(
                        out=mv2[:, t, 1:2], in0=mv2[:, t, 1:2], in1=mv[:, 1:2],
                        op=Alu.add,
                    )

                gps = s_ps.tile([128, 1024], F32, tag="sps", name="gnps")
                for t in range(2):
                    nc.tensor.matmul(
                        out=gps[:, 0:2], lhsT=GM[:, t], rhs=mv2[:, t],
                        start=(t == 0), stop=(t == 1),
                    )
                gsb = small.tile([128, 2], F32)
                nc.vector.tensor_copy(out=gsb, in_=gps[:, 0:2])
                # gstat rows 0..8: col0 = group mean, col1 = rsqrt(var+eps);
                # rows 8..128 stay zero for the padded broadcast matmul.
                gstat = small.tile([128, 2], F32)
                nc.vector.memset(gstat, 0.0)
                nc.vector.tensor_copy(out=gstat[:G, 0:1], in_=gsb[:G, 0:1])
                gvar = small.tile([G, 1], F32)
                nc.vector.tensor_tensor(
                    out=gvar, in0=gsb[:G, 0:1], in1=gsb[:G, 0:1], op=Alu.mult
                )
                nc.vector.tensor_tensor(
                    out=gvar, in0=gsb[:G, 1:2], in1=gvar, op=Alu.subtract
                )
                # rsqrt via exp(-0.5*ln(v+eps)): stays in the ln/exp table set
                nc.scalar.activation(out=gvar, in_=gvar, func=Ln, bias=EPS8)
                nc.scalar.activation(
                    out=gstat[:G, 1:2], in_=gvar, func=Exp, scale=-0.5
                )

                # broadcast group stats back to channels
                AB = []  # [t] -> [128, 2] (alpha, beta)
                for t in range(2):
                    bc = s_ps.tile([128, 1024], F32, tag="sps", name="bcst")
                    nc.tensor.matmul(out=bc[:, 0:2], lhsT=GMT[:, t], rhs=gstat)
                    bsb = small.tile([128, 2], F32, tag="bsb", name="bsb")
                    nc.vector.tensor_copy(out=bsb, in_=bc[:, 0:2])
                    ab = small.tile([128, 2], F32, tag=f"ab{t}", name=f"ab{t}")
                    # alpha = rstd * w
                    nc.vector.tensor_tensor(
                        out=ab[:, 0:1], in0=bsb[:, 1:2], in1=WN[:, t : t + 1],
                        op=Alu.mult,
                    )
                    # beta = b - mean * alpha
                    nc.vector.tensor_tensor(
                        out=ab[:, 1:2], in0=bsb[:, 0:1], in1=ab[:, 0:1],
                        op=Alu.mult,
                    )
                    nc.vector.tensor_tensor(
                        out=ab[:, 1:2], in0=BN[:, t : t + 1], in1=ab[:, 1:2],
                        op=Alu.subtract,
                    )
                    AB.append(ab)

                # ---- chunked production helpers ----
                def hq_chunk(c):  # normalized queries, 512 cols (DVE)
                    for t in range(2):
                        nc.vector.tensor_scalar(
                            out=HQ[t][:, c * 512 : (c + 1) * 512],
                            in0=XQ[t][:, c * 512 : (c + 1) * 512],
                            scalar1=AB[t][:, 0:1], scalar2=AB[t][:, 1:2],
                            op0=Alu.mult, op1=Alu.add,
                        )

                def h_chunk(c):  # normalized keys, 512 cols (gpsimd)
                    for t in range(2):
                        nc.gpsimd.tensor_scalar(
                            out=H[t][:, c * 512 : (c + 1) * 512],
                            in0=X[t][:, c * 512 : (c + 1) * 512],
                            scalar1=AB[t][:, 0:1], scalar2=AB[t][:, 1:2],
                            op0=Alu.mult, op1=Alu.add,
                        )

                def q_chunk(c):  # q projection for queries 512c.. (both t)
                    ps = s_ps.tile([128, 1024], F32, tag="sps", name="qps")
                    for t in range(2):
                        for ct in range(2):
                            nc.tensor.matmul(
                                out=ps[:, t * 512 : (t + 1) * 512],
                                lhsT=WQ[:, ct, t * 128 : (t + 1) * 128],
                                rhs=HQ[ct][:, c * 512 : (c + 1) * 512],
                                start=(ct == 0), stop=(ct == 1),
                            )
                    for t in range(2):
                        nc.vector.tensor_scalar_add(
                            out=QT[t][:, c * 512 : (c + 1) * 512],
                            in0=ps[:, t * 512 : (t + 1) * 512],
                            scalar1=BQ[:, t : t + 1],
                        )

                def k_piece(n, hp):  # K head-pair hp for keys 512n..
                    ps = s_ps.tile([128, 1024], F32, tag="sps", name="kps")
                    for h2 in range(2):
                        h = 2 * hp + h2
                        for ct in range(2):
                            nc.tensor.matmul(
                                out=ps[:, h2 * 512 : (h2 + 1) * 512],
                                lhsT=WKZ[:, ct, h],
                                rhs=H[ct][:, n * 512 : (n + 1) * 512],
                                start=(ct == 0), stop=(ct == 1),
                            )
                    for h2 in range(2):
                        h = 2 * hp + h2
                        nc.vector.tensor_scalar_add(
                            out=KZ[h][:, n * 512 : (n + 1) * 512],
                            in0=ps[:, h2 * 512 : (h2 + 1) * 512],
                            scalar1=BKZ[:, h : h + 1],
                        )

                def v_piece(n, jp):  # V^T for key tiles 4n+2jp, 4n+2jp+1
                    ps = s_ps.tile([128, 1024], F32, tag="sps", name="vps")
                    for jo in range(2):
                        j = 4 * n + 2 * jp + jo
                        for ct in range(2):
                            nc.tensor.matmul(
                                out=ps[:, jo * 512 : jo * 512 + NH * 65],
                                lhsT=H[ct][:, j * 128 : (j + 1) * 128],
                                rhs=WV[:, ct],
                                start=(ct == 0), stop=(ct == 1),
                            )
                    for jo in range(2):
                        j = 4 * n + 2 * jp + jo
                        nc.vector.tensor_tensor(
                            out=V[:, j],
                            in0=ps[:, jo * 512 : jo * 512 + NH * 65],
                            in1=VB, op=Alu.add,
                        )

                # minimal chain to the first QK: chunk 0 of HQ/H/Q/K
                hq_chunk(0)
                h_chunk(0)
                q_chunk(0)
                k_piece(0, 0)
                k_piece(0, 1)

                # ic0 production schedule: group g, slot jj -> emit tasks
                prod = {}
                for g in range(NKC):
                    prod[(g, 0)] = [lambda n=g: v_piece(n, 0)]
                    prod[(g, 1)] = [lambda n=g: v_piece(n, 1)]
                    if g + 1 < NKC:
                        prod[(g, 1)].append(lambda n=g + 1: h_chunk(n))
                        prod[(g, 2)] = [lambda n=g + 1: k_piece(n, 0)]
                        prod[(g, 3)] = [lambda n=g + 1: k_piece(n, 1)]
                    if 1 <= g <= 3:
                        prod[(g, 0)].append(lambda c=g: hq_chunk(c))
                        prod[(g, 2 if g + 1 < NKC else 0)].append(
                            lambda c=g: q_chunk(c)
                        )

                # ---- attention + projection ----
                def normalize(accs):
                    # per-head o tiles, normalized by the denominator row
                    # (acc row 0; everything stays at partition base 0)
                    OH = []
                    for h in range(NH):
                        dc = small.tile([1, 512], F32, tag="dcp", name="dcp")
                        nc.vector.tensor_copy(out=dc, in_=accs[h][0:1, :])
                        r = small.tile([1, 512], F32, tag="recip", name="recip")
                        rs = small.tile([1, 512], F32, tag="rscr", name="rscr")
                        nc.vector.reciprocal_approx_accurate(r, dc, rs)
                        R = att.tile([65, 512], F32, tag="rbc", name="rbc")
                        nc.gpsimd.partition_broadcast(R, r)
                        oh = att.tile([65, 512], BF16, tag=f"oh{h}", name=f"oh{h}")
                        nc.vector.tensor_tensor(
                            out=oh, in0=accs[h][0:65, :], in1=R, op=Alu.mult,
                        )
                        OH.append(oh)
                    return OH

                def project(OH, ic):
                    # proj + bias + residual (per-head K=65 matmuls; weight
                    # row 0 is zero, discarding the denominator row)
                    for mt in range(2):
                        pjt = s_ps.tile([128, 1024], F32, tag="sps", name="pj")
                        pj = pjt[:, 0:512]
                        for h in range(NH):
                            nc.tensor.matmul(
                                out=pj,
                                lhsT=WP[:, h, mt],
                                rhs=OH[h],
                                start=(h == 0), stop=(h == NH - 1),
                            )
                        ob = att.tile([128, 512], F32, tag="outsb", name="outsb")
                        nc.vector.tensor_scalar_add(
                            out=ob, in0=pj, scalar1=BP[:, mt : mt + 1]
                        )
                        nc.gpsimd.tensor_tensor(
                            out=ob, in0=ob,
                            in1=XQ[mt][:, ic * 512 : (ic + 1) * 512],
                            op=Alu.add,
                        )
                        nc.sync.dma_start(
                            out=out[
                                mt * 128 : (mt + 1) * 128,
                                ic * 512 : (ic + 1) * 512,
                            ],
                            in_=ob,
                        )

                pending = None  # (OH, ic) awaiting projection
                for ic in range(NIC):
                    accs = [
                        acc_ps.tile([128, 512], F32, tag=f"acc{h}", name=f"acc{h}")
                        for h in range(NH)
                    ]
                    av_q = []  # (E, j, sp) not yet fed to the AV matmuls

                    def flush_av(upto):
                        while len(av_q) > upto:
                            E, j, sp = av_q.pop(0)
                            for h2 in range(2):
                                h = 2 * sp + h2
                                nc.tensor.matmul(
                                    out=accs[h][0:65, :],
                                    lhsT=V[:, j, h * 65 : (h + 1) * 65],
                                    rhs=E[:, h2 * 512 : (h2 + 1) * 512],
                                    start=(j == 0), stop=(j == NJT - 1),
                                )

                    for n in range(NKC):
                        for jj in range(4):
                            if ic == 0:
                                for task in prod.get((n, jj), ()):
                                    task()
                            j = 4 * n + jj
                            for sp in range(2):
                                S = s_ps.tile([128, 1024], F32, tag="sps", name="sps")
                                for h2 in range(2):
                                    nc.tensor.matmul(
                                        out=S[:, h2 * 512 : (h2 + 1) * 512],
                                        lhsT=KZ[2 * sp + h2][
                                            :, j * 128 : (j + 1) * 128
                                        ],
                                        rhs=QT[sp][:, ic * 512 : (ic + 1) * 512],
                                        start=True, stop=True,
                                    )
                                E = expp.tile([128, 1024], BF16, tag="exps", name="e")
                                nc.scalar.activation(out=E, in_=S, func=Exp, scale=SCALE)
                                av_q.append((E, j, sp))
                                flush_av(AVLAG)
                        if n == 0 and pending is not None:
                            project(*pending)
                            pending = None
                    flush_av(0)
                    pending = (normalize(accs), ic)
                project(*pending)
    if finalize:
        nc.finalize()
    return nc


def _prep_weights(norm_w, norm_b, qkv_w, qkv_b, proj_w, proj_b):
    """Host-side layout (pure reshapes/transposes + dtype casts of weights)."""
    import ml_dtypes

    f = np.float32
    cdt = ml_dtypes.bfloat16

    def ctile(v):  # (256,) -> (128, 2) per channel-tile columns
        return np.ascontiguousarray(np.asarray(v).reshape(2, 128).T, dtype=f)

    def ptile(m):  # (256, N) -> (128, 2, N)
        return np.ascontiguousarray(
            np.asarray(m).reshape(2, 128, -1).transpose(1, 0, 2), dtype=f
        )

    qkv_w = np.asarray(qkv_w)
    qkv_b = np.asarray(qkv_b)
    wqT = qkv_w[:C].T  # (256, 256)
    wkT = qkv_w[C : 2 * C].T  # (256, 256) key rows
    # per-head K weights, zero-padded so each head's output occupies the same
    # 64 partition rows as its q in the packed 2-head Q tile
    wkzT = np.zeros((C, NH, 128), dtype=f)
    bkz = np.zeros((128, NH), dtype=f)
    for h in range(NH):
        off = 64 * (h % 2)
        wkzT[:, h, off : off + 64] = wkT[:, h * 64 : (h + 1) * 64]
        bkz[off : off + 64, h] = qkv_b[C + h * 64 : C + (h + 1) * 64]
    wvm = qkv_w[2 * C :]  # (256, 256)
    wvT = np.zeros((C, NH * 65), dtype=f)
    vb = np.zeros((128, NH * 65), dtype=f)
    for h in range(NH):
        wvT[:, h * 65 + 1 : h * 65 + 65] = wvm[h * 64 : (h + 1) * 64].T
        vb[:, h * 65 + 1 : h * 65 + 65] = qkv_b[
            2 * C + h * 64 : 2 * C + (h + 1) * 64
        ][None, :]
        vb[:, h * 65] = 1.0  # leading ones column -> denominator at psum row 0
    # zero-padded group masks (value 1/32 for group-mean aggregation; one-hot
    # transpose for the broadcast back to channels)
    gm = np.zeros((C, 128), dtype=f)
    for c in range(C):
        gm[c, c // 32] = 1.0 / 32.0
    # gmaskT param layout [p, t, 128]: partition p = group index (only 0..8
    # nonzero), free = channel within tile t
    gmaskT = np.zeros((128, 2, 128), dtype=f)
    for c in range(C):
        gmaskT[c // 32, c // 128, c % 128] = 1.0

    def wph(pw):  # (256 out, 256 in) -> [65, NH, 2, 128]; row 0 stays zero
        w = np.zeros((65, NH, 2, 128), dtype=f)
        for h in range(NH):
            for mt in range(2):
                w[1:, h, mt, :] = pw[
                    mt * 128 : (mt + 1) * 128, h * 64 : (h + 1) * 64
                ].T
        return w
    return dict(
        wn2=ctile(norm_w),
        bn2=ctile(norm_b),
        wq=ptile(wqT).astype(cdt),
        bq2=np.ascontiguousarray(qkv_b[:C].reshape(2, 128).T, dtype=f),
        wkz=ptile(wkzT.reshape(C, NH * 128))
        .reshape(128, 2, NH, 128)
        .astype(cdt),
        bkz=bkz,
        wv=ptile(wvT).astype(cdt),
        vb=vb,
        wproj=wph(np.asarray(proj_w)).astype(cdt),
        bproj2=ctile(proj_b),
        gmask=ptile(gm),
        gmaskT=gmaskT,
    )


_NC_CACHE = {}
_RUN_OPTS = {}  # extra kwargs for run_bass_kernel_spmd (test harness sets trace)
LAST_RESULT = None


def _get_nc():
    if "nc" not in _NC_CACHE:
        _NC_CACHE["nc"] = build()
    return _NC_CACHE["nc"]


def kernel(x, norm_w, norm_b, qkv_w, qkv_b, proj_w, proj_b, **_):
    nc = _get_nc()
    w = _prep_weights(norm_w, norm_b, qkv_w, qkv_b, proj_w, proj_b)
    x = np.asarray(x, dtype=np.float32)
    Bv, Cv, Hv, Wv = x.shape
    xf = x.reshape(Bv, Cv, Hv * Wv)
    in_maps = []
    for j in range(8):
        b, qh = j // 2, j % 2
        m = dict(w)
        m["x"] = np.ascontiguousarray(xf[b])
        m["xq"] = np.ascontiguousarray(xf[b][:, qh * Q : (qh + 1) * Q])
        in_maps.append(m)
    res = run_bass_kernel_spmd(nc, in_maps, core_ids=list(range(8)), **_RUN_OPTS)
    global LAST_RESULT
    LAST_RESULT = res
    outf = np.empty((Bv, Cv, Hv * Wv), dtype=np.float32)
    for j in range(8):
        b, qh = j // 2, j % 2
        outf[b][:, qh * Q : (qh + 1) * Q] = res.results[j]["out"]
    return outf.reshape(Bv, Cv, Hv, Wv)
